# revision 12
# baseline (speedup 1.0000x reference)
"""GAT (graph attention) kernel for Trainium2, 8-core SPMD — one head per core.

Reference computation (per head k):
    h = x @ W_k.T + b_k                       # (N, F)
    left[n]  = h[n] . a_left_k ; right[m] = h[m] . a_right_k
    e[n, m]  = leaky_relu(left[n] + right[m], 0.2)
    a        = softmax_m(where(mask[n, m], e, -1e9))
    out_k    = elu(a @ h)                      # (N, F)
Full output = concat_k(out_k)  -> (N, K*F)

v3 design (vs v2):
    - FLIPPED aggregation matmul: em chunk [128m, 128n] is the STATIONARY
      operand, [h | ones] [128m, 129] the moving one.  out[n, f] lands with
      n on partitions and the softmax denominator in PSUM column 128 — the
      separate `sums` matmul (half of all attention PE work) disappears,
      and the epilogue 1/sums is a per-partition tensor_scalar (no DRAM
      broadcast roundtrip).
    - PSUM: per-quarter accumulator [128, 8, 256] f32 = 4 banks, each
      [128, 129] block half-bank aligned; bufs=2 double-buffers quarters.
    - epilogue per quarter: recip of col 128, copy PSUM->bf16, 4x-mode
      tensor_scalar scale, ELU via hacked Exp table (scale=5).
    - output stored [n, 128] row-major; host concat, no transpose.
    - hijacked ACT `Exp` table computes exp(leaky_relu(x, 0.2)) in one
      pass as before; all PE operands bf16.
"""

import json
import os
import shutil
import tempfile

import numpy as np

import concourse.bass as bass
import concourse.tile as tile
from concourse import bacc, mybir
from concourse.bass_utils import run_bass_kernel_spmd
from concourse.masks import make_identity

N_NODES = 4096
F_IN = 512
K_HEADS = 8
F_OUT = 128
NEG_SLOPE = 0.2
N_CORES = 8

f32 = mybir.dt.float32
bf16 = mybir.dt.bfloat16


# --------------------------------------------------------------------------- #
# activation-table hack: make `exp` compute exp(leaky_relu(x, 0.2))
# --------------------------------------------------------------------------- #
def _make_hacked_act_dir(dst):
    from neuronxcc.driver.Job import Job
    from neuronxcc.driver.jobs.support.FindActInfo import findActInfoFile

    src = os.path.dirname(findActInfoFile(Job.getPackageDir(), "gen3"))
    os.makedirs(dst, exist_ok=True)
    for fn in os.listdir(src):
        shutil.copy(os.path.join(src, fn), os.path.join(dst, fn))

    info = json.load(open(os.path.join(dst, "act_info.json")))
    for s in info["act_func_sets"]:
        if "exp" not in s["act"]:
            continue
        prof = json.load(open(os.path.join(dst, s["profile_json"])))
        start = prof["func_to_bkt_start_idx"]["exp"]
        starts = sorted(prof["func_to_bkt_start_idx"].values())
        ends = [e for e in starts if e > start]
        end = ends[0] if ends else prof["bkt_entry_cnt"]

        path = os.path.join(dst, s["bkt_bin"])
        b = np.fromfile(path, dtype=np.float32).reshape(-1, 8).copy()
        sl = b[start:end]
        neg = sl[:, 4] < 0.0
        x0 = sl[neg, 4].astype(np.float64)
        g = np.exp(NEG_SLOPE * x0)
        sl[neg, 0] = g
        sl[neg, 1] = NEG_SLOPE * g
        sl[neg, 2] = NEG_SLOPE**2 * g / 2.0
        sl[neg, 3] = NEG_SLOPE**3 * g / 6.0
        b[start:end] = sl
        b.tofile(path)
    return os.path.join(dst, "act_info.json")


_ACT_DIR = None


def setup_act_tables():
    global _ACT_DIR
    if _ACT_DIR is None:
        d = os.path.join(tempfile.gettempdir(), "gat_act_tables")
        _ACT_DIR = _make_hacked_act_dir(d)
    os.environ["BASS_ACT_ROOT_JSON_PATH"] = _ACT_DIR
    return _ACT_DIR


# --------------------------------------------------------------------------- #
# bass program
# --------------------------------------------------------------------------- #
def build(n_nodes=N_NODES, n_tile=1024, num_devices=N_CORES, timing_mode=False, repeat=1,
          n_dve=9, n_gp=12):
    """One head per core. Returns compiled Bacc module.

    timing_mode: large inputs/outputs become Internal DRAM (no host traffic);
    the whole compute body is emitted `repeat` times so device time dominates
    dispatch overhead.

    n_dve: of the 32 m-chunks per quarter, how many compute em on VectorE via
    the outer-product identity exp(leaky(l+r)) = max(e^l e^r, e^{.2l} e^{.2r})
    instead of ScalarE.  n_gp: how many of the remaining ScalarE-path chunks
    run the mask multiply on GpSimd instead of VectorE."""
    setup_act_tables()

    n = n_nodes
    mc_cnt = n // 128          # m-chunks
    nq = n // n_tile           # n-range quarters
    nb_cnt = n_tile // 128     # n-blocks per quarter (stationary chunks)
    cseg = F_IN // 128         # contraction chunks for the projection
    pseg = 512                 # projection output chunk (1 PSUM bank)

    # spread the DVE-path chunks and the gpsimd-masked chunks evenly over mc
    dve_set = {int(i * mc_cnt / n_dve) for i in range(n_dve)} if n_dve else set()
    rest = [mc for mc in range(mc_cnt) if mc not in dve_set]
    gp_set = {rest[int(i * len(rest) / n_gp)] for i in range(n_gp)} if n_gp else set()

    nc = bacc.Bacc("TRN2", target_bir_lowering=False, debug=False, num_devices=num_devices)

    big_kind = "Internal" if timing_mode else "ExternalInput"
    xT_d = nc.dram_tensor("xT", [F_IN, n], bf16, kind=big_kind).ap()
    wkT_d = nc.dram_tensor("wkT", [F_IN, F_OUT], bf16, kind="ExternalInput").ap()
    bk_d = nc.dram_tensor("bk", [F_OUT, 1], f32, kind="ExternalInput").ap()
    alr_d = nc.dram_tensor("alr", [F_OUT, 2], bf16, kind="ExternalInput").ap()
    maskT_d = nc.dram_tensor("maskT", [n, n], bf16, kind=big_kind).ap()
    out_kind = "Internal" if timing_mode else "ExternalOutput"
    out_h = nc.dram_tensor("out", [n, F_OUT], bf16, kind=out_kind)
    sink_d = None
    if timing_mode:
        sink_d = nc.dram_tensor("sink", [1, 128], bf16, kind="ExternalOutput").ap()

    left_dram = nc.dram_tensor("left_scratch", [1, n], f32, kind="Internal")
    right_dram = nc.dram_tensor("right_scratch", [1, n], f32, kind="Internal")
    a_dram = nc.dram_tensor("a_scratch", [1, n], bf16, kind="Internal")
    b_dram = nc.dram_tensor("b_scratch", [1, n], bf16, kind="Internal")

    def dram_ap(handle, offset, pattern):
        return bass.AP(tensor=handle.ap().tensor, offset=offset, ap=pattern)

    with tile.TileContext(nc) as tc:
        with (
            tc.tile_pool(name="consts", bufs=1) as consts,
            tc.tile_pool(name="work", bufs=8) as work,
            tc.tile_pool(name="epi", bufs=3) as epi,
            tc.tile_pool(name="dvework", bufs=3) as dvework,
        ):
            if timing_mode:
                # fill the Internal inputs on-device: x = 0, mask = 1
                fz = consts.tile([128, n], bf16, tag="fz")
                nc.vector.memset(fz, 0.0)
                for c in range(cseg):
                    nc.sync.dma_start(out=xT_d[c * 128 : (c + 1) * 128, :], in_=fz)
                fo = consts.tile([128, n], bf16, tag="fo")
                nc.vector.memset(fo, 1.0)
                for r in range(n // 128):
                    nc.sync.dma_start(out=maskT_d[r * 128 : (r + 1) * 128, :], in_=fo)

            emitted_o_sb = [None]
            for _rep in range(repeat):
              # ---------------- phase 0: load constants ---------------- #
              xT_sb = consts.tile([128, cseg, n], bf16, tag="xT", bufs=1)
              for c in range(cseg):
                  nc.sync.dma_start(out=xT_sb[:, c, :], in_=xT_d[c * 128 : (c + 1) * 128, :])
              wkT_sb = consts.tile([128, cseg, F_OUT], bf16, tag="wkT", bufs=2)
              for c in range(cseg):
                  nc.sync.dma_start(out=wkT_sb[:, c, :], in_=wkT_d[c * 128 : (c + 1) * 128, :])
              bk_sb = consts.tile([128, 1], f32)
              nc.sync.dma_start(out=bk_sb, in_=bk_d)
              alr_sb = consts.tile([128, 2], bf16)
              nc.sync.dma_start(out=alr_sb, in_=alr_d)
              identity = consts.tile([128, 128], bf16)
              make_identity(nc, identity)

              # -------- phase 1+2 fused, pipelined in 512-col chunks --------- #
              # per chunk s: project hT -> bias-add -> lr matmul -> lr chunk to
              # DRAM -> left_bc/right_sc chunk broadcasts -> 4 transposes to
              # h1_mf (with a ones column at f=128 for the fused denominator).
              hT_sb = consts.tile([128, n], bf16, tag="hT", bufs=2)
              left_bc = consts.tile([128, n], f32, tag="leftbc", bufs=2)
              right_sc = consts.tile([128, mc_cnt], f32, tag="rightsc", bufs=2)
              h1_mf = consts.tile([128, mc_cnt, 132], bf16, tag="hmf", bufs=2)
              nc.vector.memset(h1_mf[:, :, 128:132], 1.0)
              with tc.tile_pool(name="psA", bufs=2, space="PSUM") as psA:
                  for s in range(n // pseg):
                      hT_ps = psA.tile([128, pseg], f32, tag="hp", bufs=3)
                      for c in range(cseg):
                          nc.tensor.matmul(
                              hT_ps,
                              lhsT=wkT_sb[:, c, :],
                              rhs=xT_sb[:, c, s * pseg : (s + 1) * pseg],
                              start=(c == 0),
                              stop=(c == cseg - 1),
                          )
                      nc.vector.tensor_scalar_add(
                          out=hT_sb[:, s * pseg : (s + 1) * pseg], in0=hT_ps, scalar1=bk_sb
                      )
                      # left/right chunk: lr[2, pseg] = [a_l | a_r].T @ h_T
                      lr_ps = psA.tile([2, pseg], f32, tag="lrp")
                      nc.tensor.matmul(
                          lr_ps,
                          lhsT=alr_sb,
                          rhs=hT_sb[:, s * pseg : (s + 1) * pseg],
                          start=True,
                          stop=True,
                      )
                      lr_chunk = consts.tile([2, pseg], f32, tag="lrchunk", bufs=2)
                      nc.vector.tensor_copy(out=lr_chunk, in_=lr_ps)
                      nc.sync.dma_start(
                          out=left_dram.ap()[:, s * pseg : (s + 1) * pseg],
                          in_=lr_chunk[0:1, :],
                      )
                      nc.sync.dma_start(
                          out=right_dram.ap()[:, s * pseg : (s + 1) * pseg],
                          in_=lr_chunk[1:2, :],
                      )
                      # broadcasts / reshapes of this chunk (via DRAM roundtrip)
                      nc.sync.dma_start(
                          out=left_bc[:, s * pseg : (s + 1) * pseg],
                          in_=dram_ap(left_dram, s * pseg, [[0, 128], [1, pseg]]),
                      )
                      nchunk = pseg // 128
                      nc.sync.dma_start(
                          out=right_sc[:, s * nchunk : (s + 1) * nchunk],
                          in_=dram_ap(right_dram, s * pseg, [[1, 128], [128, nchunk]]),
                      )
                      # h in [m, f] layout for this chunk's 4 m-blocks (bf16)
                      for j in range(s * nchunk, (s + 1) * nchunk):
                          tr_ps = psA.tile([128, 128], bf16, tag="tr")
                          nc.tensor.transpose(
                              tr_ps, hT_sb[:, j * 128 : (j + 1) * 128], identity
                          )
                          nc.vector.tensor_copy(out=h1_mf[:, j, 0:128], in_=tr_ps)

              # -------- phase 2b: outer-product factors for the DVE path ----- #
              # exp(leaky(l+r)) = max(e^l e^r, e^{.2l} e^{.2r}).  The hacked
              # Exp table computes exp(x) for x>=0 and exp(.2 x) for x<0, so
              # e^v = table(min(v, 5v)) and e^{.2 v} = table(min(v, .2 v)).
              A_bc = consts.tile([128, n], bf16, tag="Abc", bufs=1)
              B_bc = consts.tile([128, n], bf16, tag="Bbc", bufs=1)
              C_sc = consts.tile([128, mc_cnt], f32, tag="Csc", bufs=2)
              D_sc = consts.tile([128, mc_cnt], f32, tag="Dsc", bufs=2)
              if n_dve:
                  left_rs = consts.tile([128, mc_cnt], f32, tag="leftrs", bufs=2)
                  nc.sync.dma_start(
                      out=left_rs, in_=dram_ap(left_dram, 0, [[1, 128], [128, mc_cnt]])
                  )
                  tmp_sc = consts.tile([128, mc_cnt], f32, tag="tmpsc", bufs=2)
                  ab_sc = consts.tile([128, mc_cnt], bf16, tag="absc", bufs=2)
                  for (src, dst_dram) in ((left_rs, a_dram), (left_rs, b_dram)):
                      is_a = dst_dram is a_dram
                      nc.vector.scalar_tensor_tensor(
                          out=tmp_sc, in0=src, scalar=5.0 if is_a else NEG_SLOPE,
                          in1=src, op0=mybir.AluOpType.mult,
                          op1=mybir.AluOpType.min,
                      )
                      nc.scalar.activation(
                          out=ab_sc, in_=tmp_sc,
                          func=mybir.ActivationFunctionType.Exp, scale=1.0,
                      )
                      nc.sync.dma_start(
                          out=dram_ap(dst_dram, 0, [[1, 128], [128, mc_cnt]]),
                          in_=ab_sc,
                      )
                  nc.sync.dma_start(out=A_bc, in_=dram_ap(a_dram, 0, [[0, 128], [1, n]]))
                  nc.sync.dma_start(out=B_bc, in_=dram_ap(b_dram, 0, [[0, 128], [1, n]]))
                  tmp2_sc = consts.tile([128, mc_cnt], f32, tag="tmp2sc", bufs=2)
                  nc.vector.scalar_tensor_tensor(
                      out=tmp2_sc, in0=right_sc, scalar=5.0, in1=right_sc,
                      op0=mybir.AluOpType.mult, op1=mybir.AluOpType.min,
                  )
                  nc.scalar.activation(
                      out=C_sc, in_=tmp2_sc,
                      func=mybir.ActivationFunctionType.Exp, scale=1.0,
                  )
                  nc.vector.scalar_tensor_tensor(
                      out=tmp2_sc, in0=right_sc, scalar=NEG_SLOPE, in1=right_sc,
                      op0=mybir.AluOpType.mult, op1=mybir.AluOpType.min,
                  )
                  nc.scalar.activation(
                      out=D_sc, in_=tmp2_sc,
                      func=mybir.ActivationFunctionType.Exp, scale=1.0,
                  )

              # ---------------- phase 3: main attention loop ---------------- #
              with tc.tile_pool(name="psC", bufs=1, space="PSUM") as psC:
                  for q in range(nq):
                      n0 = q * n_tile
                      # [128, nb, 512] f32 = 8 banks; block nb's [128, 129]
                      # accumulator owns bank nb (start=True clears has_written
                      # for the whole bank, so accumulation groups must be
                      # bank-disjoint).
                      acc_ps = psC.tile([128, nb_cnt, 512], f32, tag="acc")

                      for mc in range(mc_cnt):
                          mask_sb = work.tile([128, n_tile], bf16, tag="mask")
                          nc.sync.dma_start(
                              out=mask_sb,
                              in_=maskT_d[mc * 128 : (mc + 1) * 128, n0 : n0 + n_tile],
                          )
                          em_sb = work.tile([128, n_tile], bf16, tag="em")
                          if mc in dve_set:
                              # DVE path: em = max(e^l e^r, e^{.2l} e^{.2r})
                              t1_sb = dvework.tile([128, n_tile], bf16, tag="t1")
                              nc.vector.tensor_scalar_mul(
                                  out=t1_sb,
                                  in0=A_bc[:, n0 : n0 + n_tile],
                                  scalar1=C_sc[:, mc : mc + 1],
                              )
                              u_sb = dvework.tile([128, n_tile], bf16, tag="u2")
                              nc.vector.scalar_tensor_tensor(
                                  out=u_sb,
                                  in0=B_bc[:, n0 : n0 + n_tile],
                                  scalar=D_sc[:, mc : mc + 1],
                                  in1=t1_sb,
                                  op0=mybir.AluOpType.mult,
                                  op1=mybir.AluOpType.max,
                              )
                              nc.vector.tensor_tensor(
                                  out=em_sb, in0=u_sb, in1=mask_sb,
                                  op=mybir.AluOpType.mult,
                              )
                          else:
                              # em = exp(leaky(left + right)) in ONE ScalarE
                              # pass (hacked Exp table; bias = per-part right)
                              nc.scalar.activation(
                                  out=em_sb,
                                  in_=left_bc[:, n0 : n0 + n_tile],
                                  func=mybir.ActivationFunctionType.Exp,
                                  bias=right_sc[:, mc : mc + 1],
                                  scale=1.0,
                              )
                              # em *= mask (bf16, 2x mode, in place) — on
                              # GpSimd for a subset to offload VectorE
                              eng = nc.gpsimd if mc in gp_set else nc.vector
                              eng.tensor_tensor(
                                  out=em_sb, in0=em_sb, in1=mask_sb,
                                  op=mybir.AluOpType.mult,
                              )
                          first, last = mc == 0, mc == mc_cnt - 1
                          # flipped agg: em chunk stationary, [h | 1] moving;
                          # col 128 of each acc block accumulates the softmax
                          # denominator.
                          for nb in range(nb_cnt):
                              nc.tensor.matmul(
                                  acc_ps[:, nb, 0:129],
                                  lhsT=em_sb[:, nb * 128 : (nb + 1) * 128],
                                  rhs=h1_mf[:, mc, 0:129],
                                  start=first,
                                  stop=last,
                              )

                      # ---- epilogue for this quarter (n on partitions) ---- #
                      rs_sb = epi.tile([128, nb_cnt, 1], f32, tag="rs")
                      nc.vector.reciprocal(
                          out=rs_sb, in_=acc_ps[:, :, 128:129]
                      )
                      y_sb = epi.tile([128, nb_cnt * 128], bf16, tag="y")
                      nc.vector.tensor_copy(out=y_sb, in_=acc_ps[:, :, 0:128])
                      # u = y * (1/sums): per-block per-partition scalar, 4x
                      for nb in range(nb_cnt):
                          nc.vector.tensor_scalar_mul(
                              out=y_sb[:, nb * 128 : (nb + 1) * 128],
                              in0=y_sb[:, nb * 128 : (nb + 1) * 128],
                              scalar1=rs_sb[:, nb, :],
                          )
                      # elu(u) = max(u, exp(min(u,0)) - 1)
                      # (exp of a negative via hacked table: scale=5)
                      t_sb = epi.tile([128, nb_cnt * 128], bf16, tag="t")
                      nc.vector.tensor_scalar_min(
                          out=t_sb,
                          in0=y_sb,
                          scalar1=0.0,
                      )
                      nc.scalar.activation(
                          out=t_sb,
                          in_=t_sb,
                          func=mybir.ActivationFunctionType.Exp,
                          scale=5.0,
                      )
                      o_sb = epi.tile([128, nb_cnt * 128], bf16, tag="o")
                      nc.vector.scalar_tensor_tensor(
                          out=o_sb,
                          in0=t_sb,
                          scalar=-1.0,
                          in1=y_sb,
                          op0=mybir.AluOpType.add,
                          op1=mybir.AluOpType.max,
                      )
                      # store: row (n0 + nb*128 + p), col f  <-  o_sb[p, nb*128+f]
                      nc.sync.dma_start(
                          out=dram_ap(
                              out_h,
                              n0 * F_OUT,
                              [[F_OUT, 128], [128 * F_OUT, nb_cnt], [1, F_OUT]],
                          ),
                          in_=o_sb,
                      )
                      emitted_o_sb[0] = o_sb

            if timing_mode and sink_d is not None:
                nc.sync.dma_start(out=sink_d, in_=emitted_o_sb[0][0:1, 0:128])

    nc.compile()
    return nc


# --------------------------------------------------------------------------- #
# host entry point
# --------------------------------------------------------------------------- #
_NC_CACHE = {}


def _get_nc():
    key = (N_NODES, 1024)
    if key not in _NC_CACHE:
        _NC_CACHE[key] = build(N_NODES, 1024, N_CORES)
    return _NC_CACHE[key]


def make_in_maps(x, mask, W, b, a_left, a_right):
    import ml_dtypes

    xT = np.ascontiguousarray(x.T).astype(ml_dtypes.bfloat16)
    maskT = np.ascontiguousarray(mask.T).astype(ml_dtypes.bfloat16)
    in_maps = []
    for k in range(K_HEADS):
        Wk = W[k * F_OUT : (k + 1) * F_OUT, :]
        in_maps.append(
            {
                "xT": xT,
                "wkT": np.ascontiguousarray(Wk.T).astype(ml_dtypes.bfloat16),
                "bk": np.ascontiguousarray(
                    b[k * F_OUT : (k + 1) * F_OUT].reshape(F_OUT, 1), dtype=np.float32
                ),
                "alr": np.ascontiguousarray(
                    np.stack([a_left[k], a_right[k]], axis=1)
                ).astype(ml_dtypes.bfloat16),
                "maskT": maskT,
            }
        )
    return in_maps


def kernel(x, mask, W, b, a_left, a_right):
    x = np.asarray(x)
    mask = np.asarray(mask)
    W = np.asarray(W)
    b = np.asarray(b)
    a_left = np.asarray(a_left)
    a_right = np.asarray(a_right)
    nc = _get_nc()
    in_maps = make_in_maps(x, mask, W, b, a_left, a_right)
    res = run_bass_kernel_spmd(nc, in_maps, core_ids=list(range(N_CORES)))
    outs = [
        np.ascontiguousarray(res.results[k]["out"].astype(np.float32))
        for k in range(K_HEADS)
    ]
    return np.concatenate(outs, axis=1)


if __name__ == "__main__":
    import reference as R

    inputs = {k: np.asarray(v) for k, v in R.setup_inputs().items()}
    expected = np.asarray(R.reference(**R.setup_inputs()))
    got = kernel(**inputs)
    aerr = np.abs(got - expected)
    scale = np.abs(expected).max()
    print(f"absmax err {aerr.max():.3e}  scale {scale:.3f}  rel {aerr.max() / scale:.3e}")


# revision 15
# speedup vs baseline: 1.9088x; 1.9088x over previous
"""GAT (graph attention) kernel for Trainium2, 8-core SPMD — one head per core.

Reference computation (per head k):
    h = x @ W_k.T + b_k                       # (N, F)
    left[n]  = h[n] . a_left_k ; right[m] = h[m] . a_right_k
    e[n, m]  = leaky_relu(left[n] + right[m], 0.2)
    a        = softmax_m(where(mask[n, m], e, -1e9))
    out_k    = elu(a @ h)                      # (N, F)
Full output = concat_k(out_k)  -> (N, K*F)

v3 design (vs v2):
    - FLIPPED aggregation matmul: em chunk [128m, 128n] is the STATIONARY
      operand, [h | ones] [128m, 129] the moving one.  out[n, f] lands with
      n on partitions and the softmax denominator in PSUM column 128 — the
      separate `sums` matmul (half of all attention PE work) disappears,
      and the epilogue 1/sums is a per-partition tensor_scalar (no DRAM
      broadcast roundtrip).
    - PSUM: per-quarter accumulator [128, 8, 256] f32 = 4 banks, each
      [128, 129] block half-bank aligned; bufs=2 double-buffers quarters.
    - epilogue per quarter: recip of col 128, copy PSUM->bf16, 4x-mode
      tensor_scalar scale, ELU via hacked Exp table (scale=5).
    - output stored [n, 128] row-major; host concat, no transpose.
    - hijacked ACT `Exp` table computes exp(leaky_relu(x, 0.2)) in one
      pass as before; all PE operands bf16.
"""

import json
import os
import shutil
import tempfile

import numpy as np

import concourse.bass as bass
import concourse.tile as tile
from concourse import bacc, mybir
from concourse.bass_utils import run_bass_kernel_spmd
from concourse.masks import make_identity

N_NODES = 4096
F_IN = 512
K_HEADS = 8
F_OUT = 128
NEG_SLOPE = 0.2
N_CORES = 8

f32 = mybir.dt.float32
bf16 = mybir.dt.bfloat16


# --------------------------------------------------------------------------- #
# activation-table hack: make `exp` compute exp(leaky_relu(x, 0.2))
# --------------------------------------------------------------------------- #
def _make_hacked_act_dir(dst):
    from neuronxcc.driver.Job import Job
    from neuronxcc.driver.jobs.support.FindActInfo import findActInfoFile

    src = os.path.dirname(findActInfoFile(Job.getPackageDir(), "gen3"))
    os.makedirs(dst, exist_ok=True)
    for fn in os.listdir(src):
        shutil.copy(os.path.join(src, fn), os.path.join(dst, fn))

    info = json.load(open(os.path.join(dst, "act_info.json")))
    for s in info["act_func_sets"]:
        if "exp" not in s["act"]:
            continue
        prof = json.load(open(os.path.join(dst, s["profile_json"])))
        start = prof["func_to_bkt_start_idx"]["exp"]
        starts = sorted(prof["func_to_bkt_start_idx"].values())
        ends = [e for e in starts if e > start]
        end = ends[0] if ends else prof["bkt_entry_cnt"]

        path = os.path.join(dst, s["bkt_bin"])
        b = np.fromfile(path, dtype=np.float32).reshape(-1, 8).copy()
        sl = b[start:end]
        neg = sl[:, 4] < 0.0
        x0 = sl[neg, 4].astype(np.float64)
        g = np.exp(NEG_SLOPE * x0)
        sl[neg, 0] = g
        sl[neg, 1] = NEG_SLOPE * g
        sl[neg, 2] = NEG_SLOPE**2 * g / 2.0
        sl[neg, 3] = NEG_SLOPE**3 * g / 6.0
        b[start:end] = sl
        b.tofile(path)
    return os.path.join(dst, "act_info.json")


_ACT_DIR = None


def setup_act_tables():
    global _ACT_DIR
    if _ACT_DIR is None:
        d = os.path.join(tempfile.gettempdir(), "gat_act_tables")
        _ACT_DIR = _make_hacked_act_dir(d)
    os.environ["BASS_ACT_ROOT_JSON_PATH"] = _ACT_DIR
    return _ACT_DIR


# --------------------------------------------------------------------------- #
# bass program
# --------------------------------------------------------------------------- #
def build(n_nodes=N_NODES, n_tile=1024, num_devices=N_CORES, timing_mode=False, repeat=1,
          n_dve=0, n_gp=0):
    """One head per core. Returns compiled Bacc module.

    timing_mode: large inputs/outputs become Internal DRAM (no host traffic);
    the whole compute body is emitted `repeat` times so device time dominates
    dispatch overhead.

    n_dve: of the 32 m-chunks per quarter, how many compute em on VectorE via
    the outer-product identity exp(leaky(l+r)) = max(e^l e^r, e^{.2l} e^{.2r})
    instead of ScalarE.  n_gp: how many of the remaining ScalarE-path chunks
    run the mask multiply on GpSimd instead of VectorE."""
    setup_act_tables()

    n = n_nodes
    mc_cnt = n // 128          # m-chunks
    nq = n // n_tile           # n-range quarters
    nb_cnt = n_tile // 128     # n-blocks per quarter (stationary chunks)
    cseg = F_IN // 128         # contraction chunks for the projection
    pseg = 512                 # projection output chunk (1 PSUM bank)

    # spread the DVE-path chunks and the gpsimd-masked chunks evenly over mc
    dve_set = {int(i * mc_cnt / n_dve) for i in range(n_dve)} if n_dve else set()
    rest = [mc for mc in range(mc_cnt) if mc not in dve_set]
    gp_set = {rest[int(i * len(rest) / n_gp)] for i in range(n_gp)} if n_gp else set()

    nc = bacc.Bacc("TRN2", target_bir_lowering=False, debug=False, num_devices=num_devices)

    big_kind = "Internal" if timing_mode else "ExternalInput"
    xT_d = nc.dram_tensor("xT", [F_IN, n], bf16, kind=big_kind).ap()
    wkT_d = nc.dram_tensor("wkT", [F_IN, F_OUT], bf16, kind="ExternalInput").ap()
    bk_d = nc.dram_tensor("bk", [F_OUT, 1], f32, kind="ExternalInput").ap()
    alr_d = nc.dram_tensor("alr", [F_OUT, 2], bf16, kind="ExternalInput").ap()
    maskT_d = nc.dram_tensor("maskT", [n, n], bf16, kind=big_kind).ap()
    out_kind = "Internal" if timing_mode else "ExternalOutput"
    out_h = nc.dram_tensor("out", [n, F_OUT], bf16, kind=out_kind)
    sink_d = None
    if timing_mode:
        sink_d = nc.dram_tensor("sink", [1, 128], bf16, kind="ExternalOutput").ap()

    left_dram = nc.dram_tensor("left_scratch", [1, n], f32, kind="Internal")
    right_dram = nc.dram_tensor("right_scratch", [1, n], f32, kind="Internal")
    a_dram = nc.dram_tensor("a_scratch", [1, n], bf16, kind="Internal")
    b_dram = nc.dram_tensor("b_scratch", [1, n], bf16, kind="Internal")

    def dram_ap(handle, offset, pattern):
        return bass.AP(tensor=handle.ap().tensor, offset=offset, ap=pattern)

    with tile.TileContext(nc) as tc:
        with (
            tc.tile_pool(name="consts", bufs=1) as consts,
            tc.tile_pool(name="work", bufs=8) as work,
            tc.tile_pool(name="epi", bufs=3) as epi,
            tc.tile_pool(name="dvework", bufs=3) as dvework,
        ):
            if timing_mode:
                # fill the Internal inputs on-device: x = 0, mask = 1
                fz = consts.tile([128, n], bf16, tag="fz")
                nc.vector.memset(fz, 0.0)
                for c in range(cseg):
                    nc.sync.dma_start(out=xT_d[c * 128 : (c + 1) * 128, :], in_=fz)
                fo = consts.tile([128, n], bf16, tag="fo")
                nc.vector.memset(fo, 1.0)
                for r in range(n // 128):
                    nc.sync.dma_start(out=maskT_d[r * 128 : (r + 1) * 128, :], in_=fo)

            emitted_o_sb = [None]
            for _rep in range(repeat):
              # ---------------- phase 0: load constants ---------------- #
              xT_sb = consts.tile([128, cseg, n], bf16, tag="xT", bufs=1)
              for c in range(cseg):
                  nc.sync.dma_start(out=xT_sb[:, c, :], in_=xT_d[c * 128 : (c + 1) * 128, :])
              wkT_sb = consts.tile([128, cseg, F_OUT], bf16, tag="wkT", bufs=2)
              for c in range(cseg):
                  nc.sync.dma_start(out=wkT_sb[:, c, :], in_=wkT_d[c * 128 : (c + 1) * 128, :])
              bk_sb = consts.tile([128, 1], f32)
              nc.sync.dma_start(out=bk_sb, in_=bk_d)
              alr_sb = consts.tile([128, 2], bf16)
              nc.sync.dma_start(out=alr_sb, in_=alr_d)
              identity = consts.tile([128, 128], bf16)
              make_identity(nc, identity)

              # -------- phase 1+2 fused, pipelined in 512-col chunks --------- #
              # per chunk s: project hT -> bias-add -> lr matmul -> lr chunk to
              # DRAM -> left_bc/right_sc chunk broadcasts -> 4 transposes to
              # h1_mf (with a ones column at f=128 for the fused denominator).
              hT_sb = consts.tile([128, n], bf16, tag="hT", bufs=2)
              left_bc = consts.tile([128, n], f32, tag="leftbc", bufs=2)
              right_sc = consts.tile([128, mc_cnt], f32, tag="rightsc", bufs=2)
              h1_mf = consts.tile([128, mc_cnt, 132], bf16, tag="hmf", bufs=2)
              nc.vector.memset(h1_mf[:, :, 128:132], 1.0)
              with tc.tile_pool(name="psA", bufs=2, space="PSUM") as psA:
                  for s in range(n // pseg):
                      hT_ps = psA.tile([128, pseg], f32, tag="hp", bufs=3)
                      for c in range(cseg):
                          nc.tensor.matmul(
                              hT_ps,
                              lhsT=wkT_sb[:, c, :],
                              rhs=xT_sb[:, c, s * pseg : (s + 1) * pseg],
                              start=(c == 0),
                              stop=(c == cseg - 1),
                          )
                      nc.vector.tensor_scalar_add(
                          out=hT_sb[:, s * pseg : (s + 1) * pseg], in0=hT_ps, scalar1=bk_sb
                      )
                      # left/right chunk: lr[2, pseg] = [a_l | a_r].T @ h_T
                      lr_ps = psA.tile([2, pseg], f32, tag="lrp")
                      nc.tensor.matmul(
                          lr_ps,
                          lhsT=alr_sb,
                          rhs=hT_sb[:, s * pseg : (s + 1) * pseg],
                          start=True,
                          stop=True,
                      )
                      lr_chunk = consts.tile([2, pseg], f32, tag="lrchunk", bufs=2)
                      nc.vector.tensor_copy(out=lr_chunk, in_=lr_ps)
                      nc.sync.dma_start(
                          out=left_dram.ap()[:, s * pseg : (s + 1) * pseg],
                          in_=lr_chunk[0:1, :],
                      )
                      nc.sync.dma_start(
                          out=right_dram.ap()[:, s * pseg : (s + 1) * pseg],
                          in_=lr_chunk[1:2, :],
                      )
                      # broadcasts / reshapes of this chunk (via DRAM roundtrip)
                      nc.sync.dma_start(
                          out=left_bc[:, s * pseg : (s + 1) * pseg],
                          in_=dram_ap(left_dram, s * pseg, [[0, 128], [1, pseg]]),
                      )
                      nchunk = pseg // 128
                      nc.sync.dma_start(
                          out=right_sc[:, s * nchunk : (s + 1) * nchunk],
                          in_=dram_ap(right_dram, s * pseg, [[1, 128], [128, nchunk]]),
                      )
                      # h in [m, f] layout for this chunk's 4 m-blocks (bf16)
                      for j in range(s * nchunk, (s + 1) * nchunk):
                          tr_ps = psA.tile([128, 128], bf16, tag="tr")
                          nc.tensor.transpose(
                              tr_ps, hT_sb[:, j * 128 : (j + 1) * 128], identity
                          )
                          nc.vector.tensor_copy(out=h1_mf[:, j, 0:128], in_=tr_ps)

              # -------- phase 2b: outer-product factors for the DVE path ----- #
              # exp(leaky(l+r)) = max(e^l e^r, e^{.2l} e^{.2r}).  The hacked
              # Exp table computes exp(x) for x>=0 and exp(.2 x) for x<0, so
              # e^v = table(min(v, 5v)) and e^{.2 v} = table(min(v, .2 v)).
              if n_dve:
                  A_bc = consts.tile([128, n], bf16, tag="Abc", bufs=1)
                  B_bc = consts.tile([128, n], bf16, tag="Bbc", bufs=1)
                  C_sc = consts.tile([128, mc_cnt], f32, tag="Csc", bufs=2)
                  D_sc = consts.tile([128, mc_cnt], f32, tag="Dsc", bufs=2)
                  left_rs = consts.tile([128, mc_cnt], f32, tag="leftrs", bufs=2)
                  nc.sync.dma_start(
                      out=left_rs, in_=dram_ap(left_dram, 0, [[1, 128], [128, mc_cnt]])
                  )
                  tmp_sc = consts.tile([128, mc_cnt], f32, tag="tmpsc", bufs=2)
                  ab_sc = consts.tile([128, mc_cnt], bf16, tag="absc", bufs=2)
                  for (src, dst_dram) in ((left_rs, a_dram), (left_rs, b_dram)):
                      is_a = dst_dram is a_dram
                      nc.vector.scalar_tensor_tensor(
                          out=tmp_sc, in0=src, scalar=5.0 if is_a else NEG_SLOPE,
                          in1=src, op0=mybir.AluOpType.mult,
                          op1=mybir.AluOpType.min,
                      )
                      nc.scalar.activation(
                          out=ab_sc, in_=tmp_sc,
                          func=mybir.ActivationFunctionType.Exp, scale=1.0,
                      )
                      nc.sync.dma_start(
                          out=dram_ap(dst_dram, 0, [[1, 128], [128, mc_cnt]]),
                          in_=ab_sc,
                      )
                  nc.sync.dma_start(out=A_bc, in_=dram_ap(a_dram, 0, [[0, 128], [1, n]]))
                  nc.sync.dma_start(out=B_bc, in_=dram_ap(b_dram, 0, [[0, 128], [1, n]]))
                  tmp2_sc = consts.tile([128, mc_cnt], f32, tag="tmp2sc", bufs=2)
                  nc.vector.scalar_tensor_tensor(
                      out=tmp2_sc, in0=right_sc, scalar=5.0, in1=right_sc,
                      op0=mybir.AluOpType.mult, op1=mybir.AluOpType.min,
                  )
                  nc.scalar.activation(
                      out=C_sc, in_=tmp2_sc,
                      func=mybir.ActivationFunctionType.Exp, scale=1.0,
                  )
                  nc.vector.scalar_tensor_tensor(
                      out=tmp2_sc, in0=right_sc, scalar=NEG_SLOPE, in1=right_sc,
                      op0=mybir.AluOpType.mult, op1=mybir.AluOpType.min,
                  )
                  nc.scalar.activation(
                      out=D_sc, in_=tmp2_sc,
                      func=mybir.ActivationFunctionType.Exp, scale=1.0,
                  )

              # ---------------- phase 3: main attention loop ---------------- #
              with tc.tile_pool(name="psC", bufs=1, space="PSUM") as psC:
                  for q in range(nq):
                      n0 = q * n_tile
                      # [128, nb, 512] f32 = 8 banks; block nb's [128, 129]
                      # accumulator owns bank nb (start=True clears has_written
                      # for the whole bank, so accumulation groups must be
                      # bank-disjoint).
                      acc_ps = psC.tile([128, nb_cnt, 512], f32, tag="acc")

                      for mc in range(mc_cnt):
                          mask_sb = work.tile([128, n_tile], bf16, tag="mask")
                          nc.sync.dma_start(
                              out=mask_sb,
                              in_=maskT_d[mc * 128 : (mc + 1) * 128, n0 : n0 + n_tile],
                          )
                          em_sb = work.tile([128, n_tile], bf16, tag="em")
                          if mc in dve_set:
                              # DVE path: em = max(e^l e^r, e^{.2l} e^{.2r})
                              t1_sb = dvework.tile([128, n_tile], bf16, tag="t1")
                              nc.vector.tensor_scalar_mul(
                                  out=t1_sb,
                                  in0=A_bc[:, n0 : n0 + n_tile],
                                  scalar1=C_sc[:, mc : mc + 1],
                              )
                              u_sb = dvework.tile([128, n_tile], bf16, tag="u2")
                              nc.vector.tensor_scalar_mul(
                                  out=u_sb,
                                  in0=B_bc[:, n0 : n0 + n_tile],
                                  scalar1=D_sc[:, mc : mc + 1],
                              )
                              nc.vector.tensor_tensor(
                                  out=u_sb, in0=u_sb, in1=t1_sb,
                                  op=mybir.AluOpType.max,
                              )
                              nc.vector.tensor_tensor(
                                  out=em_sb, in0=u_sb, in1=mask_sb,
                                  op=mybir.AluOpType.mult,
                              )
                          else:
                              # em = exp(leaky(left + right)) in ONE ScalarE
                              # pass (hacked Exp table; bias = per-part right)
                              nc.scalar.activation(
                                  out=em_sb,
                                  in_=left_bc[:, n0 : n0 + n_tile],
                                  func=mybir.ActivationFunctionType.Exp,
                                  bias=right_sc[:, mc : mc + 1],
                                  scale=1.0,
                              )
                              # em *= mask (bf16, 2x mode, in place) — on
                              # GpSimd for a subset to offload VectorE
                              eng = nc.gpsimd if mc in gp_set else nc.vector
                              eng.tensor_tensor(
                                  out=em_sb, in0=em_sb, in1=mask_sb,
                                  op=mybir.AluOpType.mult,
                              )
                          first, last = mc == 0, mc == mc_cnt - 1
                          # flipped agg: em chunk stationary, [h | 1] moving;
                          # col 128 of each acc block accumulates the softmax
                          # denominator.
                          for nb in range(nb_cnt):
                              nc.tensor.matmul(
                                  acc_ps[:, nb, 0:129],
                                  lhsT=em_sb[:, nb * 128 : (nb + 1) * 128],
                                  rhs=h1_mf[:, mc, 0:129],
                                  start=first,
                                  stop=last,
                              )

                      # ---- epilogue for this quarter (n on partitions) ---- #
                      rs_sb = epi.tile([128, nb_cnt, 1], f32, tag="rs")
                      nc.vector.reciprocal(
                          out=rs_sb, in_=acc_ps[:, :, 128:129]
                      )
                      y_sb = epi.tile([128, nb_cnt * 128], bf16, tag="y")
                      nc.vector.tensor_copy(out=y_sb, in_=acc_ps[:, :, 0:128])
                      # u = y * (1/sums): per-block per-partition scalar, 4x
                      for nb in range(nb_cnt):
                          nc.vector.tensor_scalar_mul(
                              out=y_sb[:, nb * 128 : (nb + 1) * 128],
                              in0=y_sb[:, nb * 128 : (nb + 1) * 128],
                              scalar1=rs_sb[:, nb, :],
                          )
                      # elu(u) = max(u, exp(min(u,0)) - 1)
                      # (exp of a negative via hacked table: scale=5)
                      t_sb = epi.tile([128, nb_cnt * 128], bf16, tag="t")
                      nc.vector.tensor_scalar_min(
                          out=t_sb,
                          in0=y_sb,
                          scalar1=0.0,
                      )
                      nc.scalar.activation(
                          out=t_sb,
                          in_=t_sb,
                          func=mybir.ActivationFunctionType.Exp,
                          scale=5.0,
                      )
                      nc.vector.tensor_scalar_add(out=t_sb, in0=t_sb, scalar1=-1.0)
                      o_sb = epi.tile([128, nb_cnt * 128], bf16, tag="o")
                      nc.vector.tensor_tensor(
                          out=o_sb, in0=t_sb, in1=y_sb, op=mybir.AluOpType.max,
                      )
                      # store: row (n0 + nb*128 + p), col f  <-  o_sb[p, nb*128+f]
                      nc.sync.dma_start(
                          out=dram_ap(
                              out_h,
                              n0 * F_OUT,
                              [[F_OUT, 128], [128 * F_OUT, nb_cnt], [1, F_OUT]],
                          ),
                          in_=o_sb,
                      )
                      emitted_o_sb[0] = o_sb

            if timing_mode and sink_d is not None:
                nc.sync.dma_start(out=sink_d, in_=emitted_o_sb[0][0:1, 0:128])

    nc.compile()
    return nc


# --------------------------------------------------------------------------- #
# host entry point
# --------------------------------------------------------------------------- #
_NC_CACHE = {}


def _get_nc():
    key = (N_NODES, 1024)
    if key not in _NC_CACHE:
        _NC_CACHE[key] = build(N_NODES, 1024, N_CORES)
    return _NC_CACHE[key]


def make_in_maps(x, mask, W, b, a_left, a_right):
    import ml_dtypes

    xT = np.ascontiguousarray(x.T).astype(ml_dtypes.bfloat16)
    maskT = np.ascontiguousarray(mask.T).astype(ml_dtypes.bfloat16)
    in_maps = []
    for k in range(K_HEADS):
        Wk = W[k * F_OUT : (k + 1) * F_OUT, :]
        in_maps.append(
            {
                "xT": xT,
                "wkT": np.ascontiguousarray(Wk.T).astype(ml_dtypes.bfloat16),
                "bk": np.ascontiguousarray(
                    b[k * F_OUT : (k + 1) * F_OUT].reshape(F_OUT, 1), dtype=np.float32
                ),
                "alr": np.ascontiguousarray(
                    np.stack([a_left[k], a_right[k]], axis=1)
                ).astype(ml_dtypes.bfloat16),
                "maskT": maskT,
            }
        )
    return in_maps


def kernel(x, mask, W, b, a_left, a_right):
    x = np.asarray(x)
    mask = np.asarray(mask)
    W = np.asarray(W)
    b = np.asarray(b)
    a_left = np.asarray(a_left)
    a_right = np.asarray(a_right)
    nc = _get_nc()
    in_maps = make_in_maps(x, mask, W, b, a_left, a_right)
    res = run_bass_kernel_spmd(nc, in_maps, core_ids=list(range(N_CORES)))
    outs = [
        np.ascontiguousarray(res.results[k]["out"].astype(np.float32))
        for k in range(K_HEADS)
    ]
    return np.concatenate(outs, axis=1)


if __name__ == "__main__":
    import reference as R

    inputs = {k: np.asarray(v) for k, v in R.setup_inputs().items()}
    expected = np.asarray(R.reference(**R.setup_inputs()))
    got = kernel(**inputs)
    aerr = np.abs(got - expected)
    scale = np.abs(expected).max()
    print(f"absmax err {aerr.max():.3e}  scale {scale:.3f}  rel {aerr.max() / scale:.3e}")


# revision 18
# speedup vs baseline: 2.0149x; 1.0556x over previous
"""GAT (graph attention) kernel for Trainium2, 8-core SPMD — one head per core.

Reference computation (per head k):
    h = x @ W_k.T + b_k                       # (N, F)
    left[n]  = h[n] . a_left_k ; right[m] = h[m] . a_right_k
    e[n, m]  = leaky_relu(left[n] + right[m], 0.2)
    a        = softmax_m(where(mask[n, m], e, -1e9))
    out_k    = elu(a @ h)                      # (N, F)
Full output = concat_k(out_k)  -> (N, K*F)

v3 design (vs v2):
    - FLIPPED aggregation matmul: em chunk [128m, 128n] is the STATIONARY
      operand, [h | ones] [128m, 129] the moving one.  out[n, f] lands with
      n on partitions and the softmax denominator in PSUM column 128 — the
      separate `sums` matmul (half of all attention PE work) disappears,
      and the epilogue 1/sums is a per-partition tensor_scalar (no DRAM
      broadcast roundtrip).
    - PSUM: per-quarter accumulator [128, 8, 256] f32 = 4 banks, each
      [128, 129] block half-bank aligned; bufs=2 double-buffers quarters.
    - epilogue per quarter: recip of col 128, copy PSUM->bf16, 4x-mode
      tensor_scalar scale, ELU via hacked Exp table (scale=5).
    - output stored [n, 128] row-major; host concat, no transpose.
    - hijacked ACT `Exp` table computes exp(leaky_relu(x, 0.2)) in one
      pass as before; all PE operands bf16.
"""

import json
import os
import shutil
import tempfile

import numpy as np

import concourse.bass as bass
import concourse.tile as tile
from concourse import bacc, mybir
from concourse.bass_utils import run_bass_kernel_spmd
from concourse.masks import make_identity

N_NODES = 4096
F_IN = 512
K_HEADS = 8
F_OUT = 128
NEG_SLOPE = 0.2
N_CORES = 8

f32 = mybir.dt.float32
bf16 = mybir.dt.bfloat16


# --------------------------------------------------------------------------- #
# activation-table hack: make `exp` compute exp(leaky_relu(x, 0.2))
# --------------------------------------------------------------------------- #
def _make_hacked_act_dir(dst):
    from neuronxcc.driver.Job import Job
    from neuronxcc.driver.jobs.support.FindActInfo import findActInfoFile

    src = os.path.dirname(findActInfoFile(Job.getPackageDir(), "gen3"))
    os.makedirs(dst, exist_ok=True)
    for fn in os.listdir(src):
        shutil.copy(os.path.join(src, fn), os.path.join(dst, fn))

    info = json.load(open(os.path.join(dst, "act_info.json")))
    for s in info["act_func_sets"]:
        if "exp" not in s["act"]:
            continue
        prof = json.load(open(os.path.join(dst, s["profile_json"])))
        start = prof["func_to_bkt_start_idx"]["exp"]
        starts = sorted(prof["func_to_bkt_start_idx"].values())
        ends = [e for e in starts if e > start]
        end = ends[0] if ends else prof["bkt_entry_cnt"]

        path = os.path.join(dst, s["bkt_bin"])
        b = np.fromfile(path, dtype=np.float32).reshape(-1, 8).copy()
        sl = b[start:end]
        neg = sl[:, 4] < 0.0
        x0 = sl[neg, 4].astype(np.float64)
        g = np.exp(NEG_SLOPE * x0)
        sl[neg, 0] = g
        sl[neg, 1] = NEG_SLOPE * g
        sl[neg, 2] = NEG_SLOPE**2 * g / 2.0
        sl[neg, 3] = NEG_SLOPE**3 * g / 6.0
        b[start:end] = sl
        b.tofile(path)
    return os.path.join(dst, "act_info.json")


_ACT_DIR = None


def setup_act_tables():
    global _ACT_DIR
    if _ACT_DIR is None:
        d = os.path.join(tempfile.gettempdir(), "gat_act_tables")
        _ACT_DIR = _make_hacked_act_dir(d)
    os.environ["BASS_ACT_ROOT_JSON_PATH"] = _ACT_DIR
    return _ACT_DIR


# --------------------------------------------------------------------------- #
# bass program
# --------------------------------------------------------------------------- #
def build(n_nodes=N_NODES, n_tile=1024, num_devices=N_CORES, timing_mode=False, repeat=1,
          n_dve=0, n_gp=0):
    """One head per core. Returns compiled Bacc module.

    timing_mode: large inputs/outputs become Internal DRAM (no host traffic);
    the whole compute body is emitted `repeat` times so device time dominates
    dispatch overhead.

    n_dve: of the 32 m-chunks per quarter, how many compute em on VectorE via
    the outer-product identity exp(leaky(l+r)) = max(e^l e^r, e^{.2l} e^{.2r})
    instead of ScalarE.  n_gp: how many of the remaining ScalarE-path chunks
    run the mask multiply on GpSimd instead of VectorE."""
    setup_act_tables()

    n = n_nodes
    mc_cnt = n // 128          # m-chunks
    nq = n // n_tile           # n-range quarters
    nb_cnt = n_tile // 128     # n-blocks per quarter (stationary chunks)
    cseg = F_IN // 128         # contraction chunks for the projection
    pseg = 512                 # projection output chunk (1 PSUM bank)

    # spread the DVE-path chunks and the gpsimd-masked chunks evenly over mc
    dve_set = {int(i * mc_cnt / n_dve) for i in range(n_dve)} if n_dve else set()
    rest = [mc for mc in range(mc_cnt) if mc not in dve_set]
    gp_set = {rest[int(i * len(rest) / n_gp)] for i in range(n_gp)} if n_gp else set()

    nc = bacc.Bacc("TRN2", target_bir_lowering=False, debug=False, num_devices=num_devices)

    big_kind = "Internal" if timing_mode else "ExternalInput"
    xT_d = nc.dram_tensor("xT", [F_IN, n], bf16, kind=big_kind).ap()
    wkT_d = nc.dram_tensor("wkT", [F_IN, F_OUT], bf16, kind="ExternalInput").ap()
    bk_d = nc.dram_tensor("bk", [F_OUT, 1], f32, kind="ExternalInput").ap()
    alr_d = nc.dram_tensor("alr", [F_OUT, 2], bf16, kind="ExternalInput").ap()
    maskT_h = nc.dram_tensor("maskT", [n, n], mybir.dt.uint8, kind=big_kind)
    maskT_d = maskT_h.ap()
    out_kind = "Internal" if timing_mode else "ExternalOutput"
    out_h = nc.dram_tensor("out", [n, F_OUT], bf16, kind=out_kind)
    sink_d = None
    if timing_mode:
        sink_d = nc.dram_tensor("sink", [1, 128], bf16, kind="ExternalOutput").ap()

    left_dram = nc.dram_tensor("left_scratch", [1, n], f32, kind="Internal")
    right_dram = nc.dram_tensor("right_scratch", [1, n], f32, kind="Internal")
    a_dram = nc.dram_tensor("a_scratch", [1, n], bf16, kind="Internal")
    b_dram = nc.dram_tensor("b_scratch", [1, n], bf16, kind="Internal")

    def dram_ap(handle, offset, pattern):
        return bass.AP(tensor=handle.ap().tensor, offset=offset, ap=pattern)

    with tile.TileContext(nc) as tc:
        with (
            tc.tile_pool(name="consts", bufs=1) as consts,
            tc.tile_pool(name="work", bufs=8) as work,
            tc.tile_pool(name="epi", bufs=3) as epi,
            tc.tile_pool(name="dvework", bufs=3) as dvework,
        ):
            if timing_mode:
                # fill the Internal inputs on-device: x = 0, mask = 1
                fz = consts.tile([128, n], bf16, tag="fz")
                nc.vector.memset(fz, 0.0)
                for c in range(cseg):
                    nc.sync.dma_start(out=xT_d[c * 128 : (c + 1) * 128, :], in_=fz)
                fo = consts.tile([128, n], mybir.dt.uint8, tag="fo")
                nc.vector.memset(fo, 1)
                for r in range(n // 128):
                    nc.sync.dma_start(out=maskT_d[r * 128 : (r + 1) * 128, :], in_=fo)

            emitted_o_sb = [None]
            for _rep in range(repeat):
              # ---------------- phase 0: load constants ---------------- #
              xT_sb = consts.tile([128, cseg, n], bf16, tag="xT", bufs=1)
              for c in range(cseg):
                  nc.sync.dma_start(out=xT_sb[:, c, :], in_=xT_d[c * 128 : (c + 1) * 128, :])
              wkT_sb = consts.tile([128, cseg, F_OUT], bf16, tag="wkT", bufs=2)
              for c in range(cseg):
                  nc.sync.dma_start(out=wkT_sb[:, c, :], in_=wkT_d[c * 128 : (c + 1) * 128, :])
              bk_sb = consts.tile([128, 1], f32)
              nc.sync.dma_start(out=bk_sb, in_=bk_d)
              alr_sb = consts.tile([128, 2], bf16)
              nc.sync.dma_start(out=alr_sb, in_=alr_d)
              identity = consts.tile([128, 128], bf16)
              make_identity(nc, identity)

              # -------- phase 1+2 fused, pipelined in 512-col chunks --------- #
              # per chunk s: project hT -> bias-add -> lr matmul -> lr chunk to
              # DRAM -> left_bc/right_sc chunk broadcasts -> 4 transposes to
              # h1_mf (with a ones column at f=128 for the fused denominator).
              hT_sb = consts.tile([128, n], bf16, tag="hT", bufs=2)
              left_bc = consts.tile([128, n], f32, tag="leftbc", bufs=2)
              right_sc = consts.tile([128, mc_cnt], f32, tag="rightsc", bufs=2)
              h1_mf = consts.tile([128, mc_cnt, 132], bf16, tag="hmf", bufs=2)
              nc.vector.memset(h1_mf[:, :, 128:132], 1.0)
              with tc.tile_pool(name="psA", bufs=2, space="PSUM") as psA:
                  for s in range(n // pseg):
                      hT_ps = psA.tile([128, pseg], f32, tag="hp", bufs=3)
                      for c in range(cseg):
                          nc.tensor.matmul(
                              hT_ps,
                              lhsT=wkT_sb[:, c, :],
                              rhs=xT_sb[:, c, s * pseg : (s + 1) * pseg],
                              start=(c == 0),
                              stop=(c == cseg - 1),
                          )
                      nc.vector.tensor_scalar_add(
                          out=hT_sb[:, s * pseg : (s + 1) * pseg], in0=hT_ps, scalar1=bk_sb
                      )
                      # left/right chunk: lr[2, pseg] = [a_l | a_r].T @ h_T
                      lr_ps = psA.tile([2, pseg], f32, tag="lrp")
                      nc.tensor.matmul(
                          lr_ps,
                          lhsT=alr_sb,
                          rhs=hT_sb[:, s * pseg : (s + 1) * pseg],
                          start=True,
                          stop=True,
                      )
                      lr_chunk = consts.tile([2, pseg], f32, tag="lrchunk", bufs=2)
                      nc.vector.tensor_copy(out=lr_chunk, in_=lr_ps)
                      nc.sync.dma_start(
                          out=left_dram.ap()[:, s * pseg : (s + 1) * pseg],
                          in_=lr_chunk[0:1, :],
                      )
                      nc.sync.dma_start(
                          out=right_dram.ap()[:, s * pseg : (s + 1) * pseg],
                          in_=lr_chunk[1:2, :],
                      )
                      # broadcasts / reshapes of this chunk (via DRAM roundtrip)
                      nc.sync.dma_start(
                          out=left_bc[:, s * pseg : (s + 1) * pseg],
                          in_=dram_ap(left_dram, s * pseg, [[0, 128], [1, pseg]]),
                      )
                      nchunk = pseg // 128
                      nc.sync.dma_start(
                          out=right_sc[:, s * nchunk : (s + 1) * nchunk],
                          in_=dram_ap(right_dram, s * pseg, [[1, 128], [128, nchunk]]),
                      )
                      # h in [m, f] layout for this chunk's 4 m-blocks (bf16)
                      for j in range(s * nchunk, (s + 1) * nchunk):
                          tr_ps = psA.tile([128, 128], bf16, tag="tr")
                          nc.tensor.transpose(
                              tr_ps, hT_sb[:, j * 128 : (j + 1) * 128], identity
                          )
                          nc.vector.tensor_copy(out=h1_mf[:, j, 0:128], in_=tr_ps)

              # -------- phase 2b: outer-product factors for the DVE path ----- #
              # exp(leaky(l+r)) = max(e^l e^r, e^{.2l} e^{.2r}).  The hacked
              # Exp table computes exp(x) for x>=0 and exp(.2 x) for x<0, so
              # e^v = table(min(v, 5v)) and e^{.2 v} = table(min(v, .2 v)).
              if n_dve:
                  A_bc = consts.tile([128, n], bf16, tag="Abc", bufs=1)
                  B_bc = consts.tile([128, n], bf16, tag="Bbc", bufs=1)
                  C_sc = consts.tile([128, mc_cnt], f32, tag="Csc", bufs=2)
                  D_sc = consts.tile([128, mc_cnt], f32, tag="Dsc", bufs=2)
                  left_rs = consts.tile([128, mc_cnt], f32, tag="leftrs", bufs=2)
                  nc.sync.dma_start(
                      out=left_rs, in_=dram_ap(left_dram, 0, [[1, 128], [128, mc_cnt]])
                  )
                  tmp_sc = consts.tile([128, mc_cnt], f32, tag="tmpsc", bufs=2)
                  ab_sc = consts.tile([128, mc_cnt], bf16, tag="absc", bufs=2)
                  for (src, dst_dram) in ((left_rs, a_dram), (left_rs, b_dram)):
                      is_a = dst_dram is a_dram
                      nc.vector.scalar_tensor_tensor(
                          out=tmp_sc, in0=src, scalar=5.0 if is_a else NEG_SLOPE,
                          in1=src, op0=mybir.AluOpType.mult,
                          op1=mybir.AluOpType.min,
                      )
                      nc.scalar.activation(
                          out=ab_sc, in_=tmp_sc,
                          func=mybir.ActivationFunctionType.Exp, scale=1.0,
                      )
                      nc.sync.dma_start(
                          out=dram_ap(dst_dram, 0, [[1, 128], [128, mc_cnt]]),
                          in_=ab_sc,
                      )
                  nc.sync.dma_start(out=A_bc, in_=dram_ap(a_dram, 0, [[0, 128], [1, n]]))
                  nc.sync.dma_start(out=B_bc, in_=dram_ap(b_dram, 0, [[0, 128], [1, n]]))
                  tmp2_sc = consts.tile([128, mc_cnt], f32, tag="tmp2sc", bufs=2)
                  nc.vector.scalar_tensor_tensor(
                      out=tmp2_sc, in0=right_sc, scalar=5.0, in1=right_sc,
                      op0=mybir.AluOpType.mult, op1=mybir.AluOpType.min,
                  )
                  nc.scalar.activation(
                      out=C_sc, in_=tmp2_sc,
                      func=mybir.ActivationFunctionType.Exp, scale=1.0,
                  )
                  nc.vector.scalar_tensor_tensor(
                      out=tmp2_sc, in0=right_sc, scalar=NEG_SLOPE, in1=right_sc,
                      op0=mybir.AluOpType.mult, op1=mybir.AluOpType.min,
                  )
                  nc.scalar.activation(
                      out=D_sc, in_=tmp2_sc,
                      func=mybir.ActivationFunctionType.Exp, scale=1.0,
                  )

              # ---------------- phase 3: main attention loop ---------------- #
              with tc.tile_pool(name="psC", bufs=1, space="PSUM") as psC:
                  for q in range(nq):
                      n0 = q * n_tile
                      # [128, nb, 512] f32 = 8 banks; block nb's [128, 129]
                      # accumulator owns bank nb (start=True clears has_written
                      # for the whole bank, so accumulation groups must be
                      # bank-disjoint).
                      acc_ps = psC.tile([128, nb_cnt, 512], f32, tag="acc")

                      for bt in range(mc_cnt // 8):
                          # one SWDGE cast-DMA (u8 -> bf16 in the SDMA
                          # datapath) brings 8 m-chunks of mask; batching
                          # amortizes the ~1us descriptor-generation cost.
                          mask_bt = work.tile([128, 8, n_tile], bf16, tag="mask", bufs=2)
                          nc.gpsimd.dma_start(
                              out=mask_bt,
                              in_=dram_ap(
                                  maskT_h,
                                  bt * 1024 * n + n0,
                                  [[n, 128], [128 * n, 8], [1, n_tile]],
                              ),
                          )
                          for pr in range(4):
                              em_pr = work.tile([128, 2, n_tile], bf16, tag="em")
                              for s in range(2):
                                  mc = bt * 8 + 2 * pr + s
                                  nc.scalar.activation(
                                      out=em_pr[:, s, :],
                                      in_=left_bc[:, n0 : n0 + n_tile],
                                      func=mybir.ActivationFunctionType.Exp,
                                      bias=right_sc[:, mc : mc + 1],
                                      scale=1.0,
                                  )
                              # em *= mask over the pair (bf16, 2x, in place)
                              nc.vector.tensor_tensor(
                                  out=em_pr, in0=em_pr,
                                  in1=mask_bt[:, 2 * pr : 2 * pr + 2, :],
                                  op=mybir.AluOpType.mult,
                              )
                              for s in range(2):
                                  mc = bt * 8 + 2 * pr + s
                                  first, last = mc == 0, mc == mc_cnt - 1
                                  for nb in range(nb_cnt):
                                      nc.tensor.matmul(
                                          acc_ps[:, nb, 0:129],
                                          lhsT=em_pr[:, s, nb * 128 : (nb + 1) * 128],
                                          rhs=h1_mf[:, mc, 0:129],
                                          start=first,
                                          stop=last,
                                      )

                              # DVE path: em = max(e^l e^r, e^{.2l} e^{.2r})

                      # ---- epilogue for this quarter (n on partitions) ---- #
                      rs_sb = epi.tile([128, nb_cnt, 1], f32, tag="rs")
                      nc.vector.reciprocal(
                          out=rs_sb, in_=acc_ps[:, :, 128:129]
                      )
                      y_sb = epi.tile([128, nb_cnt * 128], bf16, tag="y")
                      nc.scalar.copy(out=y_sb, in_=acc_ps[:, :, 0:128])
                      # u = y * (1/sums): per-block per-partition scalar, 4x
                      for nb in range(nb_cnt):
                          nc.vector.tensor_scalar_mul(
                              out=y_sb[:, nb * 128 : (nb + 1) * 128],
                              in0=y_sb[:, nb * 128 : (nb + 1) * 128],
                              scalar1=rs_sb[:, nb, :],
                          )
                      # elu(u) = max(u, exp(min(u,0)) - 1)
                      # (exp of a negative via hacked table: scale=5)
                      t_sb = epi.tile([128, nb_cnt * 128], bf16, tag="t")
                      nc.vector.tensor_scalar_min(
                          out=t_sb,
                          in0=y_sb,
                          scalar1=0.0,
                      )
                      nc.scalar.activation(
                          out=t_sb,
                          in_=t_sb,
                          func=mybir.ActivationFunctionType.Exp,
                          scale=5.0,
                      )
                      nc.vector.tensor_scalar_add(out=t_sb, in0=t_sb, scalar1=-1.0)
                      o_sb = epi.tile([128, nb_cnt * 128], bf16, tag="o")
                      nc.vector.tensor_tensor(
                          out=o_sb, in0=t_sb, in1=y_sb, op=mybir.AluOpType.max,
                      )
                      # store: row (n0 + nb*128 + p), col f  <-  o_sb[p, nb*128+f]
                      nc.sync.dma_start(
                          out=dram_ap(
                              out_h,
                              n0 * F_OUT,
                              [[F_OUT, 128], [128 * F_OUT, nb_cnt], [1, F_OUT]],
                          ),
                          in_=o_sb,
                      )
                      emitted_o_sb[0] = o_sb

            if timing_mode and sink_d is not None:
                nc.sync.dma_start(out=sink_d, in_=emitted_o_sb[0][0:1, 0:128])

    nc.compile()
    return nc


# --------------------------------------------------------------------------- #
# host entry point
# --------------------------------------------------------------------------- #
_NC_CACHE = {}


def _get_nc():
    key = (N_NODES, 1024)
    if key not in _NC_CACHE:
        _NC_CACHE[key] = build(N_NODES, 1024, N_CORES)
    return _NC_CACHE[key]


def make_in_maps(x, mask, W, b, a_left, a_right):
    import ml_dtypes

    xT = np.ascontiguousarray(x.T).astype(ml_dtypes.bfloat16)
    maskT = np.ascontiguousarray(mask.T).astype(np.uint8)
    in_maps = []
    for k in range(K_HEADS):
        Wk = W[k * F_OUT : (k + 1) * F_OUT, :]
        in_maps.append(
            {
                "xT": xT,
                "wkT": np.ascontiguousarray(Wk.T).astype(ml_dtypes.bfloat16),
                "bk": np.ascontiguousarray(
                    b[k * F_OUT : (k + 1) * F_OUT].reshape(F_OUT, 1), dtype=np.float32
                ),
                "alr": np.ascontiguousarray(
                    np.stack([a_left[k], a_right[k]], axis=1)
                ).astype(ml_dtypes.bfloat16),
                "maskT": maskT,
            }
        )
    return in_maps


def kernel(x, mask, W, b, a_left, a_right):
    x = np.asarray(x)
    mask = np.asarray(mask)
    W = np.asarray(W)
    b = np.asarray(b)
    a_left = np.asarray(a_left)
    a_right = np.asarray(a_right)
    nc = _get_nc()
    in_maps = make_in_maps(x, mask, W, b, a_left, a_right)
    res = run_bass_kernel_spmd(nc, in_maps, core_ids=list(range(N_CORES)))
    outs = [
        np.ascontiguousarray(res.results[k]["out"].astype(np.float32))
        for k in range(K_HEADS)
    ]
    return np.concatenate(outs, axis=1)


if __name__ == "__main__":
    import reference as R

    inputs = {k: np.asarray(v) for k, v in R.setup_inputs().items()}
    expected = np.asarray(R.reference(**R.setup_inputs()))
    got = kernel(**inputs)
    aerr = np.abs(got - expected)
    scale = np.abs(expected).max()
    print(f"absmax err {aerr.max():.3e}  scale {scale:.3f}  rel {aerr.max() / scale:.3e}")
